# revision 1
# baseline (speedup 1.0000x reference)
"""Trainium2 Bass kernel for a 4-D stride-1 ConvTranspose
(B=2, C=32->32, S=16^4, K=3^4, output 18^4).

Distribution: 8 cores = batch (2) x input-row chunks (p0 in 4 chunks of 4 rows).

Per core, per (p0, q1) output tile the tensor engine accumulates in PSUM
  z[(k0,o), p0, q1, q2, q3] = sum_{i,k3, valid k1, k2} w[i,o,k0,k1,k2,k3]
                              * xs[(k3,i), p0, q1-k1, p2, q3]|_{q2 = p2+k2}
with contraction dim (k3,i) = 96 (three k3-shifted copies of x on partitions,
host-prepared), k0 packed into the PSUM partition dim (out = (k0,o) = 96),
k1 scatter-style (taps with out-of-range p1 = q1-k1 skipped), and k2 handled
by writing each tap to its shifted q2-window of the PSUM tile (per-element
has_written semantics make partial tap coverage accumulate correctly).

z is copied to SBUF as bf16 (VectorE), the three k0 partition groups are
DMA-remapped to a base-0 stack (matmuls with operands at partition base != 0
die on this HW/compiler path), and a single 96-contraction identity matmul
per 486-wide chunk folds k0 into y (ScalarE/VectorE evacuate PSUM -> SBUF -> DMA).
Fold chunks are interleaved 1:1 between main-matmul tiles to hide them.

The host prepares xs/weight layouts, sums the 2-row overlaps between adjacent
p0 chunks (the distribution seam), and adds the bias.
"""

import numpy as np
import ml_dtypes

B, CIN, COUT = 2, 32, 32
S, KT = 16, 3
Q = S + KT - 1            # 18
P0C = 4                   # input rows per core
NCORES = 8
FREE = Q * Q              # 324, main matmul free dim (q2,q3)
ZROW = Q * FREE           # 5832 z elements per p0-row per partition
XROW = S * S * Q          # 4608 xs elements per p0-row per partition
FCH = 486                 # fold chunk width (12 chunks per z row)

_CACHE = {}


def _build_nc():
    import os
    import concourse.bass as bass
    import concourse.mybir as mybir
    from concourse.tile import TileContext

    ZPS_BUFS = int(os.environ.get("ZPS_BUFS", "4"))
    YPS_BUFS = int(os.environ.get("YPS_BUFS", "4"))
    NO_FOLD = os.environ.get("NO_FOLD", "") == "1"
    ZCOPY_SPLIT = os.environ.get("ZCOPY_SPLIT", "") == "1"

    bf16 = mybir.dt.bfloat16
    f32 = mybir.dt.float32

    nc = bass.Bass()
    xs_d = nc.declare_dram_parameter("xs", [96, P0C * XROW], bf16, isOutput=False)
    wf_d = nc.declare_dram_parameter("wf", [96, 9 * 96], bf16, isOutput=False)
    id_d = nc.declare_dram_parameter("ident", [96, 32], bf16, isOutput=False)
    y_d = nc.declare_dram_parameter("y", [32, 6 * ZROW], f32, isOutput=True)

    with TileContext(nc) as tc:
        with (
            tc.tile_pool(name="const", bufs=1) as cpool,
            tc.tile_pool(name="xsp", bufs=1) as xspool,
            tc.tile_pool(name="zsbp", bufs=P0C) as zpool,
            tc.tile_pool(name="ysbp", bufs=3) as ypool,
            tc.tile_pool(name="zfp", bufs=4) as zfpool,
            tc.tile_pool(name="zpsp", bufs=ZPS_BUFS, space="PSUM") as zps_pool,
            tc.tile_pool(name="ypsp", bufs=YPS_BUFS, space="PSUM") as yps_pool,
        ):
            wf_sb = cpool.tile([96, 9 * 96], bf16)
            nc.sync.dma_start(out=wf_sb[:, :], in_=wf_d[:, :])
            id_sb = cpool.tile([96, 32], bf16)
            nc.sync.dma_start(out=id_sb[:, :], in_=id_d[:, :])

            xs_sb = xspool.tile([96, P0C * XROW], bf16)
            H = XROW // 4
            for p0 in range(P0C):
                for h in range(4):
                    nc.gpsimd.dma_start(
                        out=xs_sb[:, p0 * XROW + h * H:p0 * XROW + (h + 1) * H],
                        in_=xs_d[:, p0 * XROW + h * H:p0 * XROW + (h + 1) * H],
                    )
            # [96, p0, p1(16), p2(16), q3(18)]
            xs_v = xs_sb.rearrange("p (r a b c) -> p r a b c", r=P0C, a=S, b=S, c=Q)

            z_rows = []
            zf_map = {}
            y_map = {}
            ready_folds = []
            TD = 6 * FREE          # third of a z row (6 q1-tiles = 4 FCH chunks)

            def emit_remap(q0l, td):
                # HW rejects matmuls with operands at partition base != 0, so
                # DMA the valid k0 z slices (third td) into a base-0 stack.
                ks = [k0 for k0 in range(KT) if 0 <= q0l - k0 < P0C]
                if ks == [0]:
                    zf_map[(q0l, td)] = (z_rows[q0l], td * TD, 1)
                else:
                    zf = zfpool.tile([96, TD], bf16)
                    zf_map[(q0l, td)] = (zf, 0, len(ks))
                    for j, k0 in enumerate(ks):
                        _eng = nc.sync if j % 2 else nc.gpsimd
                        _eng.dma_start(
                            out=zf[32 * j:32 * (j + 1), :],
                            in_=z_rows[q0l - k0][
                                32 * k0:32 * (k0 + 1), td * TD:(td + 1) * TD
                            ],
                        )
                for c in range(TD // FCH):
                    ready_folds.append((q0l, td, c))

            def emit_fold_chunk(q0l, td, c):
                zf, off, nk = zf_map[(q0l, td)]
                if (q0l, td) not in y_map:
                    y_map[(q0l, td)] = ypool.tile(
                        [32, TD], f32, name=f"ysb{q0l}_{td}", tag="ysb"
                    )
                y_sb = y_map[(q0l, td)]
                y_ps = yps_pool.tile([32, FCH], f32)
                nc.tensor.matmul(
                    y_ps[:, :],
                    id_sb[0:32 * nk, :],
                    zf[0:32 * nk, off + c * FCH:off + (c + 1) * FCH],
                    start=True,
                    stop=True,
                )
                if q0l >= 3 and c % 2 == 1:
                    nc.vector.tensor_copy(
                        out=y_sb[:, c * FCH:(c + 1) * FCH], in_=y_ps[:, :]
                    )
                else:
                    nc.scalar.copy(y_sb[:, c * FCH:(c + 1) * FCH], y_ps[:, :])
                if c == TD // FCH - 1:
                    nc.sync.dma_start(
                        out=y_d[:, q0l * ZROW + td * TD:q0l * ZROW + (td + 1) * TD],
                        in_=y_sb[:, :],
                    )

            def pop_fold():
                if ready_folds:
                    emit_fold_chunk(*ready_folds.pop(0))

            for p0 in range(P0C):
                z_sb = zpool.tile([96, ZROW], bf16)
                z_rows.append(z_sb)
                for q1 in range(Q):
                    ks1 = [k1 for k1 in range(KT) if 0 <= q1 - k1 < S]
                    z_ps = zps_pool.tile([96, FREE], f32)
                    z_pv = z_ps.rearrange("p (a b) -> p a b", a=Q, b=Q)
                    ntap = len(ks1) * KT
                    ti = 0
                    for k1 in ks1:
                        for k2 in range(KT):
                            t = k1 * KT + k2
                            nc.tensor.matmul(
                                z_pv[:, k2:k2 + S, :],
                                wf_sb[:, 96 * t:96 * (t + 1)],
                                xs_v[:, p0, q1 - k1, :, :],
                                start=(ti == 0),
                                stop=(ti == ntap - 1),
                            )
                            ti += 1
                    nc.vector.tensor_copy(
                        out=z_sb[:, q1 * FREE:(q1 + 1) * FREE], in_=z_ps[:, :]
                    )
                    if q1 % 6 == 5:
                        td = q1 // 6
                        # folds whose last-needed row is p0 become remappable
                        qs = [p0] if p0 < P0C - 1 else [3, 4, 5]
                        for q0l in qs:
                            emit_remap(q0l, td)
                    pop_fold()
                    if p0 == P0C - 1:
                        pop_fold()
            while ready_folds:
                pop_fold()

    _split_drain_waits(nc)
    return nc


def _split_drain_waits(nc, max_waits=1):
    """walrus CoreV3 codegen rejects instructions carrying multiple sem waits
    ("Too many sync wait commands"); hoist extras onto preceding
    single-wait NoOp instructions on the same engine."""
    import concourse.mybir as mybir

    for f in nc.m.functions:
        for b in f.blocks:
            out = []
            changed = False
            for inst in b.instructions:
                si = inst.sync_info
                if si is not None and len(si.on_wait) > max_waits:
                    waits = list(si.on_wait)
                    for k, w in enumerate(waits[:-max_waits]):
                        nd = mybir.InstNoOp(
                            name=f"{inst.name}-wsplit{k}", ins=[], outs=[]
                        )
                        nd.engine = inst.engine
                        nd.sync_info = mybir.SyncInfo(on_wait=[w], on_update=[])
                        nc.register_instruction(nd, overwrite=True)
                        out.append(nd)
                    inst.sync_info = mybir.SyncInfo(
                        on_wait=waits[-max_waits:], on_update=list(si.on_update)
                    )
                    changed = True
                out.append(inst)
            if changed:
                b.instructions = out


def _prep_host(x, weight):
    bf = ml_dtypes.bfloat16
    # xs[n, k3, i, p0, p1, p2, q3] = x[n, i, p0, p1, p2, q3-k3]
    xs = np.zeros((B, 3, CIN, S, S, S, Q), dtype=bf)
    xb = x.astype(bf)
    for k3 in range(3):
        xs[:, k3, :, :, :, :, k3:k3 + 16] = xb
    # wf[t=(k1,k2), (k3,i), (k0,o)] = w[i,o,k0,k1,k2,k3]
    # (axes 1 and 2 are scatter-style: moving reads x[p1=q1-k1, p2] and the
    #  matmul writes the q2 = p2+k2 window, so tap indices are direct.)
    wf = np.empty((9, 96, 96), dtype=bf)
    for k1 in range(3):
        for k2 in range(3):
            blk = weight[:, :, :, k1, k2, :]                # (i,o,k0,k3)
            wf[k1 * 3 + k2] = blk.transpose(3, 0, 2, 1).reshape(96, 96).astype(bf)
    wf = np.ascontiguousarray(wf.transpose(1, 0, 2)).reshape(96, 9 * 96)  # col block t
    ident = np.tile(np.eye(32, dtype=bf), (3, 1))            # (96,32)
    return xs, wf, ident


def kernel(x, weight, bias):
    from concourse.bass_utils import run_bass_kernel_spmd

    x = np.asarray(x)
    weight = np.asarray(weight)
    bias = np.asarray(bias, np.float32)

    if "nc" not in _CACHE:
        _CACHE["nc"] = _build_nc()
    nc = _CACHE["nc"]

    xs, wf, ident = _prep_host(x, weight)

    in_maps = []
    for core in range(NCORES):
        n, c = divmod(core, P0C)
        xs_core = np.ascontiguousarray(
            xs[n, :, :, P0C * c:P0C * (c + 1)]
        ).reshape(96, P0C * XROW)  # rows=(k3,i), free=(p0,p1,a2,q3)
        in_maps.append({"xs": xs_core, "wf": wf, "ident": ident})

    res = run_bass_kernel_spmd(nc, in_maps, list(range(NCORES)))

    y = np.zeros((B, COUT, Q, Q, Q, Q), np.float32)
    for core in range(NCORES):
        n, c = divmod(core, P0C)
        yc = res.results[core]["y"].reshape(32, 6, Q, Q, Q)
        y[n, :, P0C * c:P0C * c + 6] += yc
    y += bias.reshape(1, -1, 1, 1, 1, 1)
    return y

